# revision 18
# baseline (speedup 1.0000x reference)
"""Sliding-window causal GQA self-attention (RoPE + QK-RMSNorm) for TRN2.

Sharding (8 cores): core c = (b, g) with b = c // 4, g = c % 4.
Each core handles batch b, q-heads [4g, 4g+4), kv-head g, and the
column-slice [512g, 512g+512) of the c_proj contraction (row-sharded Wo).
Host sums the 4 partial outputs per batch (the "all-reduce").

v2 changes vs the fp32r baseline:
  - bf16 matmul operands everywhere (same PE rate, half the DMA bytes,
    2-4x DVE rate on elementwise ops); PSUM accumulation stays fp32.
  - DMAs spread across engine queues (sync=wqkv, vector=x tiles,
    scalar=cos/sin/masks, gpsimd=Wo prefetch + half the output stores)
    so the first matmul starts ~2us in instead of ~39us.
  - softmax denominator matmul uses a [128,128] all-ones lhsT so the
    denominator lands on every PSUM partition; evacuation is then a
    single reciprocal_approx_fast + multiply on DVE (no [1,512]
    single-lane reciprocal, no DRAM broadcast round-trip).
  - P1 RMS sum-of-squares moved to the Scalar engine via activation
    Square + accum_out (frees DVE, the P1 critical path).
  - P2 software-pipelined across heads: head h+1's scores/exp are
    emitted between head h's PV groups so PE never waits on the
    denominator evacuation or the ACT exp chain.
"""

import math
import os
import sys

sys.path.insert(0, "/opt/trn_rl_repo")

import ml_dtypes
import numpy as np

import concourse.bass as bass
import concourse.mybir as mybir
import concourse.tile as tile
from concourse import bacc
from concourse.bass_utils import run_bass_kernel_spmd

F32 = mybir.dt.float32
BF16 = mybir.dt.bfloat16
AF = mybir.ActivationFunctionType
ALU = mybir.AluOpType

B, T, C = 2, 2048, 2048
N_HEAD, N_KV_HEAD, D = 16, 4, 128
NT = T // 128          # 16 token tiles
KT = C // 128          # 16 contraction tiles
NH = N_HEAD // 4       # 4 q heads per core
FEAT = NH * D + 2 * D  # 768 projected features per core (q 512 | k 128 | v 128)
EPS = float(np.finfo(np.float32).eps)
BF = ml_dtypes.bfloat16


def _window_masks(W: int):
    """Per block-offset o = qi - kj: multiplicative mask [k, q] or None if all-valid."""
    omax = max(0, -(-W // 128))  # ceil(W/128)
    k = np.arange(128)[:, None]
    q = np.arange(128)[None, :]
    masks = {}
    for o in range(omax + 1):
        d = q + 128 * o - k
        m = ((d >= 0) & (d <= W)).astype(np.float32)
        if not np.all(m == 1.0):
            masks[o] = m
    return omax, masks


def build_nc(W: int):
    omax, masks = _window_masks(W)
    mask_off = sorted(masks.keys())
    mask_idx = {o: i for i, o in enumerate(mask_off)}
    nm = max(1, len(mask_off))

    nc = bacc.Bacc(None, target_bir_lowering=False)
    xT = nc.dram_tensor("xT", [C, T], BF16, kind="ExternalInput")
    wT = nc.dram_tensor("wT", [C, FEAT], BF16, kind="ExternalInput")
    woT = nc.dram_tensor("woT", [NH * D, C], BF16, kind="ExternalInput")
    cosb = nc.dram_tensor("cosb", [T, 64], BF16, kind="ExternalInput")
    sinb = nc.dram_tensor("sinb", [T, 64], BF16, kind="ExternalInput")
    maskd = nc.dram_tensor("maskd", [nm, 128, 128], BF16, kind="ExternalInput")
    onesd = nc.dram_tensor("onesd", [128, 128], BF16, kind="ExternalInput")
    identd = nc.dram_tensor("identd", [128, 128], BF16, kind="ExternalInput")
    outp = nc.dram_tensor("outp", [T, C], BF16, kind="ExternalOutput")

    scale = 1.0 / math.sqrt(D)

    with tile.TileContext(nc) as tc:
        with tc.tile_pool(name="persist", bufs=1) as per:
            # persistent SBUF state
            cos_sb = per.tile([128, NT, 64], BF16, tag="cos")
            sin_sb = per.tile([128, NT, 64], BF16, tag="sin")
            mask_sb = per.tile([128, nm, 128], BF16, tag="mask")
            ones_sb = per.tile([128, 128], BF16, tag="ones")
            ident_sb = per.tile([128, 128], BF16, tag="ident")
            qdT = per.tile([128, NH, T], BF16, tag="qdT")   # [d, h, tok]
            kdT = per.tile([128, T], BF16, tag="kdT")       # [d, tok]
            vsb = per.tile([128, NT, 128], BF16, tag="v")   # [tok%128, kj, d]
            yT = per.tile([128, NH, T], BF16, tag="yT")     # [d, h, tok]
            wo_sb = per.tile([128, NH, C], BF16, tag="wo")  # [d, h, out]

            # weights stream split across the sync and scalar queues,
            # k-tile granular so the first matmul only waits for ~0.2MB
            wqkv = per.tile([128, KT, FEAT], BF16, tag="wqkv")
            for k in range(KT):
                eng = nc.sync if k % 2 == 0 else nc.scalar
                eng.dma_start(
                    out=wqkv[:, k, :], in_=wT[k * 128:(k + 1) * 128, :]
                )
            # small constants follow on the scalar queue (cos/sin needed
            # first, by tile 0's rope at ~5us)
            nc.scalar.dma_start(out=cos_sb, in_=cosb.rearrange("(i p) d -> p i d", p=128))
            nc.scalar.dma_start(out=sin_sb, in_=sinb.rearrange("(i p) d -> p i d", p=128))
            nc.scalar.dma_start(out=ident_sb, in_=identd[:, :])
            nc.scalar.dma_start(out=mask_sb, in_=maskd.rearrange("m k q -> k m q"))
            nc.scalar.dma_start(out=ones_sb, in_=onesd[:, :])

            # scoresT + exp + boundary-mask emitter, shared by P1 (early
            # head-0 tiles ride the P1 ACT slack) and P2
            max_span = (omax + 1) * 128
            n_bank = (max_span + 511) // 512
            ets = {}
            _p2e_cm = tc.tile_pool(name="p2e", bufs=2 * NT)
            p2e = _p2e_cm.__enter__()

            def scores_head(h, kjs, ps_pool):
                for kj in kjs:
                    qlo = kj
                    qhi = min(kj + omax, NT - 1)
                    w = (qhi - qlo + 1) * 128
                    # quasi-equal pieces, one per psum bank
                    n_p = (w + 511) // 512
                    s = w // n_p
                    ps_s = ps_pool.tile([128, n_bank, 512], F32, tag="sT")
                    for p in range(n_p):
                        nc.tensor.matmul(
                            ps_s[:, p, 0:s],
                            kdT[:, kj * 128:(kj + 1) * 128],
                            qdT[:, h, qlo * 128 + p * s: qlo * 128 + (p + 1) * s],
                            start=True, stop=True,
                        )
                    et = p2e.tile([128, max_span], BF16, tag="expT")
                    nc.scalar.activation(
                        et[:, 0:w].rearrange("p (a b) -> p a b", a=n_p),
                        ps_s[:, 0:n_p, 0:s], AF.Exp, scale=scale)
                    for o in mask_off:
                        if qlo + o <= qhi:
                            sl = et[:, o * 128:(o + 1) * 128]
                            nc.vector.tensor_mul(
                                sl, sl, mask_sb[:, mask_idx[o], :]
                            )
                    ets[(h, kj)] = et

            # ---------------- Phase 1: QKV + RoPE + RMS + transpose ----------
            with tc.tile_pool(name="p1x", bufs=3) as p1x, \
                 tc.tile_pool(name="p1s", bufs=2) as p1s, \
                 tc.tile_pool(name="p1ps", bufs=2, space="PSUM") as p1ps, \
                 tc.tile_pool(name="p1pt", bufs=1, space="PSUM") as p1pt, \
                 tc.tile_pool(name="p2pse", bufs=1, space="PSUM") as p2pse:

                # lag-1 software pipeline: tile i's transpose+store is emitted
                # after tile i+1's matmuls so PE never waits on the DVE/ACT
                # rope/rms chain.
                pending = {}

                def p1_tail(j):
                    qn_j = pending.pop(j)
                    pt = p1pt.tile([128, NH + 1, 128], BF16, tag="pt")
                    for h in range(NH + 1):
                        nc.tensor.transpose(pt[:, h, :], qn_j[:, h, :], ident_sb)
                        dst = qdT[:, h, j * 128:(j + 1) * 128] if h < NH \
                            else kdT[:, j * 128:(j + 1) * 128]
                        # alternate evacuation engine to balance DVE/ACT
                        if h % 2 == 0:
                            nc.vector.tensor_copy(dst, pt[:, h, :])
                        else:
                            nc.scalar.activation(dst, pt[:, h, :], AF.Copy)

                for i in range(NT):
                    xk = p1x.tile([128, KT, 128], BF16, tag="xk")
                    nc.gpsimd.dma_start(
                        out=xk,
                        in_=xT[:, i * 128:(i + 1) * 128]
                        .rearrange("(kt p) t -> p kt t", p=128),
                    )
                    ps_qkv = p1ps.tile([128, 768], F32, tag="psqkv")
                    ps_q = ps_qkv[:, 0:512]
                    ps_kv = ps_qkv[:, 512:768]
                    for k in range(KT):
                        nc.tensor.matmul(
                            ps_q, xk[:, k, :], wqkv[:, k, 0:512],
                            start=(k == 0), stop=(k == KT - 1),
                        )
                    for k in range(KT):
                        nc.tensor.matmul(
                            ps_kv, xk[:, k, :], wqkv[:, k, 512:768],
                            start=(k == 0), stop=(k == KT - 1),
                        )
                    if i > 0:
                        p1_tail(i - 1)
                        if i - 1 >= 8:
                            # head-0 scores for ready kj tiles ride P1's
                            # ACT slack, shrinking P2's exp critical path
                            scores_head(0, [i - 9], p2pse)

                    # evacuate PSUM to bf16 SBUF so the rope chain runs at
                    # the 4x DVE rate
                    qsb = p1s.tile([128, 512], BF16, tag="qsb")
                    nc.scalar.activation(qsb, ps_q, AF.Copy)
                    ksb = p1s.tile([128, 128], BF16, tag="ksb")
                    nc.vector.tensor_copy(ksb, ps_kv[:, 0:128])
                    nc.vector.tensor_copy(vsb[:, i, :], ps_kv[:, 128:256])

                    # RoPE: rot1 = x1*cos + x2*sin ; rot2 = x2*cos - x1*sin
                    rot = p1s.tile([128, NH + 1, 128], BF16, tag="rot")
                    ta = p1s.tile([128, NH, 64], BF16, tag="ta")
                    tb = p1s.tile([128, NH, 64], BF16, tag="tb")
                    cos4 = cos_sb[:, i, :].unsqueeze(1).broadcast_to([128, NH, 64])
                    sin4 = sin_sb[:, i, :].unsqueeze(1).broadcast_to([128, NH, 64])
                    q3 = qsb.rearrange("p (h d) -> p h d", h=NH)
                    x1, x2 = q3[:, :, 0:64], q3[:, :, 64:128]
                    rq = rot[:, 0:NH, :]
                    nc.vector.tensor_mul(ta, x1, cos4)
                    nc.vector.tensor_mul(tb, x2, sin4)
                    nc.vector.tensor_add(rq[:, :, 0:64], ta, tb)
                    nc.vector.tensor_mul(ta, x2, cos4)
                    nc.vector.tensor_mul(tb, x1, sin4)
                    nc.vector.tensor_sub(rq[:, :, 64:128], ta, tb)
                    k1, k2 = ksb[:, 0:64], ksb[:, 64:128]
                    ka = ta[:, 0, :]
                    kb = tb[:, 0, :]
                    rk = rot[:, NH, :]
                    nc.vector.tensor_mul(ka, k1, cos_sb[:, i, :])
                    nc.vector.tensor_mul(kb, k2, sin_sb[:, i, :])
                    nc.vector.tensor_add(rk[0:128, 0:64], ka, kb)
                    nc.vector.tensor_mul(ka, k2, cos_sb[:, i, :])
                    nc.vector.tensor_mul(kb, k1, sin_sb[:, i, :])
                    nc.vector.tensor_sub(rk[0:128, 64:128], ka, kb)

                    # RMS norm: rs = 1/sqrt(mean(rot^2) + eps); the
                    # sum-of-squares runs on ACT via Square + accum_out
                    sqs = p1s.tile([128, 128], BF16, tag="sqs")
                    ss = p1s.tile([128, 8], F32, tag="ss")
                    for h in range(NH + 1):
                        nc.scalar.activation(
                            sqs, rot[:, h, :], AF.Square,
                            accum_out=ss[:, h:h + 1],
                        )
                    tt = p1s.tile([128, 8], F32, tag="tt")
                    nc.vector.tensor_scalar(
                        out=tt[:, 0:NH + 1], in0=ss[:, 0:NH + 1],
                        scalar1=1.0 / D, scalar2=EPS,
                        op0=ALU.mult, op1=ALU.add,
                    )
                    rr = p1s.tile([128, 8], F32, tag="rr")
                    nc.vector.reciprocal(rr[:, 0:NH + 1], tt[:, 0:NH + 1])
                    rs = p1s.tile([128, 8], F32, tag="rs")
                    nc.scalar.activation(rs[:, 0:NH + 1], rr[:, 0:NH + 1], AF.Sqrt)
                    qn = p1s.tile([128, NH + 1, 128], BF16, tag="qn")
                    for h in range(NH + 1):
                        nc.vector.tensor_scalar_mul(
                            qn[:, h, :], rot[:, h, :], rs[:, h:h + 1])
                    pending[i] = qn
                p1_tail(NT - 1)
                scores_head(0, [7], p2pse)

            # Wo prefetch (only needed in P3; overlaps P2)
            nc.gpsimd.dma_start(
                out=wo_sb, in_=woT.rearrange("(h p) o -> p h o", p=128)
            )

            # ---------------- Phase 2: windowed attention ---------------------
            with tc.tile_pool(name="p2s", bufs=2) as p2s, \
                 tc.tile_pool(name="p2ps", bufs=2, space="PSUM") as p2ps, \
                 tc.tile_pool(name="p2po", bufs=1, space="PSUM") as p2po, \
                 tc.tile_pool(name="p2pd", bufs=1, space="PSUM") as p2pd:

                scores_head(0, range(8, NT), p2ps)
                for h in range(NH):
                    for g in range(NT // 4):
                        ps_o = p2po.tile([128, 512], F32, tag="o")
                        ps_d = p2pd.tile([128, 512], F32, tag="d")
                        pieces = []
                        for kj in range(max(0, 4 * g - omax), min(NT - 1, 4 * g + 3) + 1):
                            lo = max(4 * g, kj)
                            hi = min(4 * g + 3, kj + omax, NT - 1)
                            if lo > hi:
                                continue
                            pieces.append((kj, lo, hi))
                        # hardware tracks has_written per element: first matmul
                        # clears the bank, later ones overwrite untouched cols
                        # and accumulate the rest — overlapping pieces are legal
                        plan = [(kj, lo, hi, idx == 0)
                                for idx, (kj, lo, hi) in enumerate(pieces)]
                        for idx, (kj, lo, hi, st) in enumerate(plan):
                            n = (hi - lo + 1) * 128
                            to = (lo - kj) * 128
                            po = (lo - 4 * g) * 128
                            last = idx == len(plan) - 1
                            nc.tensor.matmul(
                                ps_o[:, po:po + n], vsb[:, kj, :],
                                ets[(h, kj)][:, to:to + n],
                                start=st, stop=last, skip_group_check=True,
                            )
                        for idx, (kj, lo, hi, st) in enumerate(plan):
                            n = (hi - lo + 1) * 128
                            to = (lo - kj) * 128
                            po = (lo - 4 * g) * 128
                            last = idx == len(plan) - 1
                            nc.tensor.matmul(
                                ps_d[:, po:po + n], ones_sb,
                                ets[(h, kj)][:, to:to + n],
                                start=st, stop=last, skip_group_check=True,
                            )
                        # keep PE fed while this group's denominator evacuates
                        if h + 1 < NH:
                            scores_head(h + 1, range(4 * g, 4 * g + 4), p2ps)
                        inv = p2s.tile([128, 512], F32, tag="inv")
                        nc.vector.reciprocal_approx_fast(inv, ps_d)
                        nc.vector.tensor_mul(
                            yT[:, h, g * 512:(g + 1) * 512], ps_o, inv
                        )
                        # kj is dead once no later group's window reaches it
                        for kj in range(max(0, 4 * g - omax), 4 * g + 4 - omax):
                            ets.pop((h, kj), None)

            _p2e_cm.__exit__(None, None, None)

            # ---------------- Phase 3: c_proj partial -------------------------
            with tc.tile_pool(name="p3o", bufs=4) as p3o, \
                 tc.tile_pool(name="p3ps", bufs=3, space="PSUM") as p3ps:
                for og in range(C // 512):
                    for i in range(NT):
                        ps = p3ps.tile([128, 512], F32, tag="po")
                        for h in range(NH):
                            nc.tensor.matmul(
                                ps, yT[:, h, i * 128:(i + 1) * 128],
                                wo_sb[:, h, og * 512:(og + 1) * 512],
                                start=(h == 0), stop=(h == NH - 1),
                            )
                        ot = p3o.tile([128, 512], BF16, tag="ot")
                        # alternate copy engines so neither DVE nor ACT
                        # gates the PE accumulation pipeline
                        if i % 2 == 0:
                            nc.vector.tensor_copy(ot, ps)
                        else:
                            nc.scalar.activation(ot, ps, AF.Copy)
                        eng = (nc.sync, nc.gpsimd, nc.scalar)[i % 3]
                        eng.dma_start(
                            out=outp[i * 128:(i + 1) * 128, og * 512:(og + 1) * 512],
                            in_=ot,
                        )

    nc.compile()
    return nc, mask_off, nm


_CACHE = {}


def _get_nc(W: int):
    if W not in _CACHE:
        _CACHE[W] = build_nc(W)
    return _CACHE[W]


def kernel(x, cos, sin, Wq, Wk, Wv, Wo, window_left):
    x = np.asarray(x, dtype=np.float32)
    cos = np.asarray(cos, dtype=np.float32).reshape(T, 64)
    sin = np.asarray(sin, dtype=np.float32).reshape(T, 64)
    Wq = np.asarray(Wq, dtype=np.float32)
    Wk = np.asarray(Wk, dtype=np.float32)
    Wv = np.asarray(Wv, dtype=np.float32)
    Wo = np.asarray(Wo, dtype=np.float32)
    W = int(np.asarray(window_left))

    nc, mask_off, nm = _get_nc(W)
    _, mask_arrs = _window_masks(W)
    masks_np = np.zeros((nm, 128, 128), dtype=np.float32)
    for i, o in enumerate(mask_off):
        masks_np[i] = mask_arrs[o]

    ones_np = np.ones((128, 128), dtype=BF)
    ident_np = np.eye(128, dtype=np.float32).astype(BF)

    xTs = [np.ascontiguousarray(x[b].T).astype(BF) for b in range(B)]
    in_maps = []
    for c in range(8):
        b, g = c // 4, c % 4
        wcat = np.concatenate(
            [Wq[512 * g:512 * (g + 1)], Wk[128 * g:128 * (g + 1)],
             Wv[128 * g:128 * (g + 1)]], axis=0
        )  # [768, 2048]
        in_maps.append({
            "xT": xTs[b],
            "wT": np.ascontiguousarray(wcat.T).astype(BF),
            "woT": np.ascontiguousarray(Wo[:, 512 * g:512 * (g + 1)].T).astype(BF),
            "cosb": cos.astype(BF),
            "sinb": sin.astype(BF),
            "maskd": masks_np.astype(BF),
            "onesd": ones_np,
            "identd": ident_np,
        })

    trace = os.environ.get("KERNEL_TRACE") == "1"
    try:
        res = run_bass_kernel_spmd(nc, in_maps, core_ids=list(range(8)),
                                   trace=trace)
    except ModuleNotFoundError:
        # NTFF profile hook unavailable in this container — run untraced
        res = run_bass_kernel_spmd(nc, in_maps, core_ids=list(range(8)))
    global LAST_EXEC_NS
    LAST_EXEC_NS = res.exec_time_ns
    out = np.zeros((B, T, C), dtype=np.float32)
    for c in range(8):
        out[c // 4] += res.results[c]["outp"].astype(np.float32)
    return out


LAST_EXEC_NS = None


# revision 26
# speedup vs baseline: 1.0556x; 1.0556x over previous
"""Sliding-window causal GQA self-attention (RoPE + QK-RMSNorm) for TRN2.

Sharding (8 cores): core c = (b, g) with b = c // 4, g = c % 4.
Each core handles batch b, q-heads [4g, 4g+4), kv-head g, and the
column-slice [512g, 512g+512) of the c_proj contraction (row-sharded Wo).
Host sums the 4 partial outputs per batch (the "all-reduce").

v2 changes vs the fp32r baseline:
  - bf16 matmul operands everywhere (same PE rate, half the DMA bytes,
    2-4x DVE rate on elementwise ops); PSUM accumulation stays fp32.
  - DMAs spread across engine queues (sync=wqkv, vector=x tiles,
    scalar=cos/sin/masks, gpsimd=Wo prefetch + half the output stores)
    so the first matmul starts ~2us in instead of ~39us.
  - softmax denominator matmul uses a [128,128] all-ones lhsT so the
    denominator lands on every PSUM partition; evacuation is then a
    single reciprocal_approx_fast + multiply on DVE (no [1,512]
    single-lane reciprocal, no DRAM broadcast round-trip).
  - P1 RMS sum-of-squares moved to the Scalar engine via activation
    Square + accum_out (frees DVE, the P1 critical path).
  - P2 software-pipelined across heads: head h+1's scores/exp are
    emitted between head h's PV groups so PE never waits on the
    denominator evacuation or the ACT exp chain.
"""

import math
import os
import sys

sys.path.insert(0, "/opt/trn_rl_repo")

import ml_dtypes
import numpy as np

import concourse.bass as bass
import concourse.mybir as mybir
import concourse.tile as tile
from concourse import bacc
from concourse.bass_utils import run_bass_kernel_spmd

F32 = mybir.dt.float32
BF16 = mybir.dt.bfloat16
AF = mybir.ActivationFunctionType
ALU = mybir.AluOpType

B, T, C = 2, 2048, 2048
N_HEAD, N_KV_HEAD, D = 16, 4, 128
NT = T // 128          # 16 token tiles
KT = C // 128          # 16 contraction tiles
NH = N_HEAD // 4       # 4 q heads per core
FEAT = NH * D + 2 * D  # 768 projected features per core (q 512 | k 128 | v 128)
EPS = float(np.finfo(np.float32).eps)
BF = ml_dtypes.bfloat16


def _window_masks(W: int):
    """Per block-offset o = qi - kj: multiplicative mask [k, q] or None if all-valid."""
    omax = max(0, -(-W // 128))  # ceil(W/128)
    k = np.arange(128)[:, None]
    q = np.arange(128)[None, :]
    masks = {}
    for o in range(omax + 1):
        d = q + 128 * o - k
        m = ((d >= 0) & (d <= W)).astype(np.float32)
        if not np.all(m == 1.0):
            masks[o] = m
    return omax, masks


def build_nc(W: int):
    omax, masks = _window_masks(W)
    mask_off = sorted(masks.keys())
    mask_idx = {o: i for i, o in enumerate(mask_off)}
    nm = max(1, len(mask_off))

    nc = bacc.Bacc(None, target_bir_lowering=False)
    xT = nc.dram_tensor("xT", [C, T], BF16, kind="ExternalInput")
    wT = nc.dram_tensor("wT", [C, FEAT], BF16, kind="ExternalInput")
    woT = nc.dram_tensor("woT", [NH * D, C], BF16, kind="ExternalInput")
    cosb = nc.dram_tensor("cosb", [T, 64], BF16, kind="ExternalInput")
    sinb = nc.dram_tensor("sinb", [T, 64], BF16, kind="ExternalInput")
    maskd = nc.dram_tensor("maskd", [nm, 128, 128], BF16, kind="ExternalInput")
    onesd = nc.dram_tensor("onesd", [128, 128], BF16, kind="ExternalInput")
    identd = nc.dram_tensor("identd", [128, 128], BF16, kind="ExternalInput")
    outp = nc.dram_tensor("outp", [T, C], BF16, kind="ExternalOutput")

    scale = 1.0 / math.sqrt(D)

    with tile.TileContext(nc) as tc:
        with tc.tile_pool(name="persist", bufs=1) as per:
            # persistent SBUF state
            cos_sb = per.tile([128, NT, 64], BF16, tag="cos")
            sin_sb = per.tile([128, NT, 64], BF16, tag="sin")
            mask_sb = per.tile([128, nm, 128], BF16, tag="mask")
            ones_sb = per.tile([128, 128], BF16, tag="ones")
            ident_sb = per.tile([128, 128], BF16, tag="ident")
            qdT = per.tile([128, NH, T], BF16, tag="qdT")   # [d, h, tok]
            kdT = per.tile([128, T], BF16, tag="kdT")       # [d, tok]
            vsb = per.tile([128, NT, 128], BF16, tag="v")   # [tok%128, kj, d]
            yT = per.tile([128, NH, T], BF16, tag="yT")     # [d, h, tok]
            wo_sb = per.tile([128, NH, C], BF16, tag="wo")  # [d, h, out]

            # weights stream on the sync queue, k-tile granular so the first
            # matmul only waits for ~0.2MB. The scalar (ACT) queue carries NO
            # DMAs: a dma_start occupies the issuing engine for the whole
            # transfer, and ACT runs the per-tile copy/square chain.
            wqkv = per.tile([128, KT, FEAT], BF16, tag="wqkv")
            for k in range(KT):
                nc.sync.dma_start(
                    out=wqkv[:, k, :], in_=wT[k * 128:(k + 1) * 128, :]
                )

            # scoresT + exp + boundary-mask emitter, shared by P1 (early
            # head-0 tiles ride the P1 ACT slack) and P2
            max_span = (omax + 1) * 128
            n_bank = (max_span + 511) // 512
            ets = {}
            _p2e_cm = tc.tile_pool(name="p2e", bufs=2 * NT)
            p2e = _p2e_cm.__enter__()

            def scores_head(h, kjs, ps_pool):
                for kj in kjs:
                    qlo = kj
                    qhi = min(kj + omax, NT - 1)
                    w = (qhi - qlo + 1) * 128
                    # quasi-equal pieces, one per psum bank
                    n_p = (w + 511) // 512
                    s = w // n_p
                    ps_s = ps_pool.tile([128, n_bank, 512], F32, tag="sT")
                    for p in range(n_p):
                        nc.tensor.matmul(
                            ps_s[:, p, 0:s],
                            kdT[:, kj * 128:(kj + 1) * 128],
                            qdT[:, h, qlo * 128 + p * s: qlo * 128 + (p + 1) * s],
                            start=True, stop=True,
                        )
                    et = p2e.tile([128, max_span], BF16, tag="expT")
                    nc.scalar.activation(
                        et[:, 0:w].rearrange("p (a b) -> p a b", a=n_p),
                        ps_s[:, 0:n_p, 0:s], AF.Exp, scale=scale)
                    for o in mask_off:
                        if qlo + o <= qhi:
                            sl = et[:, o * 128:(o + 1) * 128]
                            nc.vector.tensor_mul(
                                sl, sl, mask_sb[:, mask_idx[o], :]
                            )
                    ets[(h, kj)] = et

            # ---------------- Phase 1: QKV + RoPE + RMS + transpose ----------
            with tc.tile_pool(name="p1x", bufs=3) as p1x, \
                 tc.tile_pool(name="p1s", bufs=2) as p1s, \
                 tc.tile_pool(name="p1ps", bufs=2, space="PSUM") as p1ps, \
                 tc.tile_pool(name="p1pt", bufs=4, space="PSUM") as p1pt:

                # lag-1 software pipeline: tile i's transpose+store is emitted
                # after tile i+1's matmuls so PE never waits on the DVE/ACT
                # rope/rms chain.
                pending = {}

                def p1_tail(j):
                    qn_j = pending.pop(j)
                    for h in range(NH + 1):
                        pt = p1pt.tile([128, 128], BF16, tag="pt")
                        nc.tensor.transpose(pt, qn_j[:, h, :], ident_sb)
                        dst = qdT[:, h, j * 128:(j + 1) * 128] if h < NH \
                            else kdT[:, j * 128:(j + 1) * 128]
                        # alternate evacuation engine to balance DVE/ACT
                        if h % 2 == 0:
                            nc.vector.tensor_copy(dst, pt)
                        else:
                            nc.scalar.activation(dst, pt, AF.Copy)

                for i in range(NT):
                    xk = p1x.tile([128, KT, 128], BF16, tag="xk")
                    nc.gpsimd.dma_start(
                        out=xk,
                        in_=xT[:, i * 128:(i + 1) * 128]
                        .rearrange("(kt p) t -> p kt t", p=128),
                    )
                    # constants ride the gpsimd queue between x tiles, each
                    # landing just before its first consumer
                    if i == 0:
                        nc.gpsimd.dma_start(
                            out=cos_sb, in_=cosb.rearrange("(i p) d -> p i d", p=128))
                        nc.gpsimd.dma_start(
                            out=sin_sb, in_=sinb.rearrange("(i p) d -> p i d", p=128))
                    elif i == 1:
                        nc.gpsimd.dma_start(out=ident_sb, in_=identd[:, :])
                        nc.gpsimd.dma_start(
                            out=mask_sb, in_=maskd.rearrange("m k q -> k m q"))
                        nc.gpsimd.dma_start(out=ones_sb, in_=onesd[:, :])
                    ps_qkv = p1ps.tile([128, 768], F32, tag="psqkv")
                    ps_q = ps_qkv[:, 0:512]
                    ps_kv = ps_qkv[:, 512:768]
                    for k in range(KT):
                        nc.tensor.matmul(
                            ps_q, xk[:, k, :], wqkv[:, k, 0:512],
                            start=(k == 0), stop=(k == KT - 1),
                        )
                    for k in range(KT):
                        nc.tensor.matmul(
                            ps_kv, xk[:, k, :], wqkv[:, k, 512:768],
                            start=(k == 0), stop=(k == KT - 1),
                        )
                    if i > 0:
                        p1_tail(i - 1)

                    # evacuate PSUM to bf16 SBUF so the rope chain runs at
                    # the 4x DVE rate
                    qsb = p1s.tile([128, 512], BF16, tag="qsb")
                    nc.scalar.activation(qsb, ps_q, AF.Copy)
                    ksb = p1s.tile([128, 128], BF16, tag="ksb")
                    nc.vector.tensor_copy(ksb, ps_kv[:, 0:128])
                    nc.vector.tensor_copy(vsb[:, i, :], ps_kv[:, 128:256])

                    # RoPE: rot1 = x1*cos + x2*sin ; rot2 = x2*cos - x1*sin
                    rot = p1s.tile([128, NH + 1, 128], BF16, tag="rot")
                    ta = p1s.tile([128, NH, 64], BF16, tag="ta")
                    tb = p1s.tile([128, NH, 64], BF16, tag="tb")
                    cos4 = cos_sb[:, i, :].unsqueeze(1).broadcast_to([128, NH, 64])
                    sin4 = sin_sb[:, i, :].unsqueeze(1).broadcast_to([128, NH, 64])
                    q3 = qsb.rearrange("p (h d) -> p h d", h=NH)
                    x1, x2 = q3[:, :, 0:64], q3[:, :, 64:128]
                    rq = rot[:, 0:NH, :]
                    nc.vector.tensor_mul(ta, x1, cos4)
                    nc.vector.tensor_mul(tb, x2, sin4)
                    nc.vector.tensor_add(rq[:, :, 0:64], ta, tb)
                    nc.vector.tensor_mul(ta, x2, cos4)
                    nc.vector.tensor_mul(tb, x1, sin4)
                    nc.vector.tensor_sub(rq[:, :, 64:128], ta, tb)
                    k1, k2 = ksb[:, 0:64], ksb[:, 64:128]
                    ka = ta[:, 0, :]
                    kb = tb[:, 0, :]
                    rk = rot[:, NH, :]
                    nc.vector.tensor_mul(ka, k1, cos_sb[:, i, :])
                    nc.vector.tensor_mul(kb, k2, sin_sb[:, i, :])
                    nc.vector.tensor_add(rk[0:128, 0:64], ka, kb)
                    nc.vector.tensor_mul(ka, k2, cos_sb[:, i, :])
                    nc.vector.tensor_mul(kb, k1, sin_sb[:, i, :])
                    nc.vector.tensor_sub(rk[0:128, 64:128], ka, kb)

                    # RMS norm: rs = 1/sqrt(mean(rot^2) + eps); the
                    # sum-of-squares runs on ACT via Square + accum_out
                    sqs = p1s.tile([128, 128], BF16, tag="sqs")
                    ss = p1s.tile([128, 8], F32, tag="ss")
                    for h in range(NH + 1):
                        nc.scalar.activation(
                            sqs, rot[:, h, :], AF.Square,
                            accum_out=ss[:, h:h + 1],
                        )
                    tt = p1s.tile([128, 8], F32, tag="tt")
                    nc.vector.tensor_scalar(
                        out=tt[:, 0:NH + 1], in0=ss[:, 0:NH + 1],
                        scalar1=1.0 / D, scalar2=EPS,
                        op0=ALU.mult, op1=ALU.add,
                    )
                    rr = p1s.tile([128, 8], F32, tag="rr")
                    nc.vector.reciprocal(rr[:, 0:NH + 1], tt[:, 0:NH + 1])
                    rs = p1s.tile([128, 8], F32, tag="rs")
                    nc.scalar.activation(rs[:, 0:NH + 1], rr[:, 0:NH + 1], AF.Sqrt)
                    qn = p1s.tile([128, NH + 1, 128], BF16, tag="qn")
                    for h in range(NH + 1):
                        nc.vector.tensor_scalar_mul(
                            qn[:, h, :], rot[:, h, :], rs[:, h:h + 1])
                    pending[i] = qn
                p1_tail(NT - 1)

            # Wo prefetch (only needed in P3; overlaps P2)
            nc.gpsimd.dma_start(
                out=wo_sb, in_=woT.rearrange("(h p) o -> p h o", p=128)
            )

            # ---------------- Phase 2: windowed attention ---------------------
            with tc.tile_pool(name="p2s", bufs=2) as p2s, \
                 tc.tile_pool(name="p2ps", bufs=2, space="PSUM") as p2ps, \
                 tc.tile_pool(name="p2po", bufs=1, space="PSUM") as p2po, \
                 tc.tile_pool(name="p2pd", bufs=1, space="PSUM") as p2pd:

                scores_head(0, range(NT), p2ps)
                for h in range(NH):
                    for g in range(NT // 4):
                        ps_o = p2po.tile([128, 512], F32, tag="o")
                        ps_d = p2pd.tile([128, 512], F32, tag="d")
                        pieces = []
                        for kj in range(max(0, 4 * g - omax), min(NT - 1, 4 * g + 3) + 1):
                            lo = max(4 * g, kj)
                            hi = min(4 * g + 3, kj + omax, NT - 1)
                            if lo > hi:
                                continue
                            pieces.append((kj, lo, hi))
                        # hardware tracks has_written per element: first matmul
                        # clears the bank, later ones overwrite untouched cols
                        # and accumulate the rest — overlapping pieces are legal
                        plan = [(kj, lo, hi, idx == 0)
                                for idx, (kj, lo, hi) in enumerate(pieces)]
                        for idx, (kj, lo, hi, st) in enumerate(plan):
                            n = (hi - lo + 1) * 128
                            to = (lo - kj) * 128
                            po = (lo - 4 * g) * 128
                            last = idx == len(plan) - 1
                            nc.tensor.matmul(
                                ps_o[:, po:po + n], vsb[:, kj, :],
                                ets[(h, kj)][:, to:to + n],
                                start=st, stop=last, skip_group_check=True,
                            )
                        for idx, (kj, lo, hi, st) in enumerate(plan):
                            n = (hi - lo + 1) * 128
                            to = (lo - kj) * 128
                            po = (lo - 4 * g) * 128
                            last = idx == len(plan) - 1
                            nc.tensor.matmul(
                                ps_d[:, po:po + n], ones_sb,
                                ets[(h, kj)][:, to:to + n],
                                start=st, stop=last, skip_group_check=True,
                            )
                        # keep PE fed while this group's denominator evacuates
                        if h + 1 < NH:
                            scores_head(h + 1, range(4 * g, 4 * g + 4), p2ps)
                        inv = p2s.tile([128, 512], F32, tag="inv")
                        nc.vector.reciprocal_approx_fast(inv, ps_d)
                        nc.vector.tensor_mul(
                            yT[:, h, g * 512:(g + 1) * 512], ps_o, inv
                        )
                        # kj is dead once no later group's window reaches it
                        for kj in range(max(0, 4 * g - omax), 4 * g + 4 - omax):
                            ets.pop((h, kj), None)

            _p2e_cm.__exit__(None, None, None)

            # ---------------- Phase 3: c_proj partial -------------------------
            with tc.tile_pool(name="p3o", bufs=4) as p3o, \
                 tc.tile_pool(name="p3ps", bufs=3, space="PSUM") as p3ps:
                for og in range(C // 512):
                    for i in range(NT):
                        ps = p3ps.tile([128, 512], F32, tag="po")
                        for h in range(NH):
                            nc.tensor.matmul(
                                ps, yT[:, h, i * 128:(i + 1) * 128],
                                wo_sb[:, h, og * 512:(og + 1) * 512],
                                start=(h == 0), stop=(h == NH - 1),
                            )
                        ot = p3o.tile([128, 512], BF16, tag="ot")
                        # alternate copy engines so neither DVE nor ACT
                        # gates the PE accumulation pipeline
                        if i % 2 == 0:
                            nc.vector.tensor_copy(ot, ps)
                        else:
                            nc.scalar.activation(ot, ps, AF.Copy)
                        eng = nc.sync if i % 2 == 0 else nc.gpsimd
                        eng.dma_start(
                            out=outp[i * 128:(i + 1) * 128, og * 512:(og + 1) * 512],
                            in_=ot,
                        )

    nc.compile()
    return nc, mask_off, nm


_CACHE = {}


def _get_nc(W: int):
    if W not in _CACHE:
        _CACHE[W] = build_nc(W)
    return _CACHE[W]


def kernel(x, cos, sin, Wq, Wk, Wv, Wo, window_left):
    x = np.asarray(x, dtype=np.float32)
    cos = np.asarray(cos, dtype=np.float32).reshape(T, 64)
    sin = np.asarray(sin, dtype=np.float32).reshape(T, 64)
    Wq = np.asarray(Wq, dtype=np.float32)
    Wk = np.asarray(Wk, dtype=np.float32)
    Wv = np.asarray(Wv, dtype=np.float32)
    Wo = np.asarray(Wo, dtype=np.float32)
    W = int(np.asarray(window_left))

    nc, mask_off, nm = _get_nc(W)
    _, mask_arrs = _window_masks(W)
    masks_np = np.zeros((nm, 128, 128), dtype=np.float32)
    for i, o in enumerate(mask_off):
        masks_np[i] = mask_arrs[o]

    ones_np = np.ones((128, 128), dtype=BF)
    ident_np = np.eye(128, dtype=np.float32).astype(BF)

    xTs = [np.ascontiguousarray(x[b].T).astype(BF) for b in range(B)]
    in_maps = []
    for c in range(8):
        b, g = c // 4, c % 4
        wcat = np.concatenate(
            [Wq[512 * g:512 * (g + 1)], Wk[128 * g:128 * (g + 1)],
             Wv[128 * g:128 * (g + 1)]], axis=0
        )  # [768, 2048]
        in_maps.append({
            "xT": xTs[b],
            "wT": np.ascontiguousarray(wcat.T).astype(BF),
            "woT": np.ascontiguousarray(Wo[:, 512 * g:512 * (g + 1)].T).astype(BF),
            "cosb": cos.astype(BF),
            "sinb": sin.astype(BF),
            "maskd": masks_np.astype(BF),
            "onesd": ones_np,
            "identd": ident_np,
        })

    trace = os.environ.get("KERNEL_TRACE") == "1"
    try:
        res = run_bass_kernel_spmd(nc, in_maps, core_ids=list(range(8)),
                                   trace=trace)
    except ModuleNotFoundError:
        # NTFF profile hook unavailable in this container — run untraced
        res = run_bass_kernel_spmd(nc, in_maps, core_ids=list(range(8)))
    global LAST_EXEC_NS
    LAST_EXEC_NS = res.exec_time_ns
    out = np.zeros((B, T, C), dtype=np.float32)
    for c in range(8):
        out[c // 4] += res.results[c]["outp"].astype(np.float32)
    return out


LAST_EXEC_NS = None
